# revision 1
# baseline (speedup 1.0000x reference)
"""Column-wise RMS normalization on 8 Trainium2 NeuronCores.

Computes y = x * rsqrt(sum(x*x, axis=0) + eps) for x [32768, 2048] f32.

Sharding: column-parallel — each core owns a contiguous block of 256
columns, making the per-column sum-of-squares entirely core-local (no
collectives). Within a core the shard is viewed as [128 p, 256 t, 256 c]
(row = p*256 + t) so every DMA moves >=8KB contiguous runs per partition.

Single-read strategy: the f32 shard is DMA'd from HBM exactly once,
cast to fp16 on the fly (SWDGE cast DMA) into a persistent SBUF cache
(16MB/core). Pass A squares the cache (DVE) and reduces over partitions
with TensorE ones-matmuls into PSUM; the scale rsqrt(u+eps) is computed
via ACT Sqrt + DVE fast reciprocal and broadcast to all partitions with
a K=1 matmul. Pass B multiplies the cached fp16 x by the broadcast scale
(DVE) and DMAs f32 results out. HBM traffic = 32MB in + 32MB out per
core, the roofline floor.
"""

import numpy as np

import concourse.bacc as bacc
import concourse.bass as bass
import concourse.tile as tile
from concourse import mybir
from concourse.bass_utils import run_bass_kernel_spmd

N, D = 32768, 2048
EPS = 1e-6
NCORES = 8
C = D // NCORES  # 256 columns per core
P = 128          # partitions
T = N // P       # 256 rows per partition
G = 8            # row-group (t) per DMA / compute chunk
NG = T // G      # 32 groups

_NC = None


def _build() -> bass.Bass:
    nc = bacc.Bacc("TRN2", target_bir_lowering=False, enable_partition_id=False)
    x = nc.dram_tensor("x", [N, C], mybir.dt.float32, kind="ExternalInput")
    y = nc.dram_tensor("y", [N, C], mybir.dt.float32, kind="ExternalOutput")
    xv = x[:, :].rearrange("(p t) c -> p t c", p=P)
    yv = y[:, :].rearrange("(p t) c -> p t c", p=P)

    with tile.TileContext(nc) as tc:
        with (
            tc.tile_pool(name="cache", bufs=1) as cachep,
            tc.tile_pool(name="consts", bufs=1) as consts,
            tc.tile_pool(name="sq", bufs=2) as sqp,
            tc.tile_pool(name="outs", bufs=4) as outp,
            tc.tile_pool(name="scale", bufs=1) as scalep,
            tc.tile_pool(name="ps", bufs=1, space="PSUM") as psp,
        ):
            xc = cachep.tile([P, T, C], mybir.dt.float16)
            ones_col = consts.tile([P, 1], mybir.dt.float16)
            nc.vector.memset(ones_col, 1.0)
            ones_row = consts.tile([1, P], mybir.dt.float32)
            nc.vector.memset(ones_row, 1.0)
            eps_t = consts.tile([P, 1], mybir.dt.float32)
            nc.vector.memset(eps_t, EPS)

            # u_ps holds 2 interleaved partial column-sum vectors (even/odd t)
            u_ps = psp.tile([1, 2 * C], mybir.dt.float32)
            s_ps = psp.tile([P, 1, C], mybir.dt.float32)

            # Pass A: cast-DMA f32->fp16 into the persistent cache (SWDGE),
            # square on DVE, reduce over partitions (PE ones-matmul
            # accumulate into PSUM). 2MB cast-DMAs for the bulk, one 1MB
            # group, then a short G=2 tail so the final square->matmul
            # chain into the scale computation is short.
            GI = 2 * G
            in_groups = (
                [(j * GI, GI) for j in range(T // GI - 1)]
                + [(T - GI, G)]
                + [(T - G + 2 * h, 2) for h in range(G // 2)]
            )
            nmm = T // 2
            k = 0
            for t0, g in in_groups:
                ts_ = slice(t0, t0 + g)
                nc.gpsimd.dma_start(out=xc[:, ts_, :], in_=xv[:, ts_, :])
                # Tail (g==2) squares get their own 4-deep slot set so they
                # don't stall on PE consuming the big groups' sq slots.
                if g > 2:
                    sq = sqp.tile([P, g, C], mybir.dt.float16, tag="sq", bufs=2)
                else:
                    sq = sqp.tile([P, g, C], mybir.dt.float16, tag="sqt", bufs=4)
                nc.vector.tensor_mul(sq, xc[:, ts_, :], xc[:, ts_, :])
                for h in range(g // 2):
                    rhs = sq[:, 2 * h : 2 * h + 2, :].rearrange("p t c -> p (t c)")
                    nc.tensor.matmul(
                        u_ps[:, :],
                        lhsT=ones_col[:, :],
                        rhs=rhs,
                        start=(k == 0),
                        stop=(k == nmm - 1),
                    )
                    k += 1

            # Scale: u = even+odd partials; s = 1/sqrt(u+eps) computed on the
            # narrow [1, C] vector, THEN broadcast to all partitions with a
            # K=1 matmul into PSUM. Pass-B muls read s straight from PSUM,
            # which keeps the post-broadcast hop off the critical path.
            u_sb = scalep.tile([1, C], mybir.dt.float32)
            upair = u_ps[:, :].rearrange("p (t c) -> p c t", t=2)
            nc.vector.reduce_sum(u_sb, upair, axis=mybir.AxisListType.X)
            tsq = scalep.tile([1, C], mybir.dt.float32)
            nc.scalar.activation(
                out=tsq[:, :],
                in_=u_sb[:, :],
                func=mybir.ActivationFunctionType.Sqrt,
                bias=eps_t[0:1, :],
                scale=1.0,
            )
            s1 = scalep.tile([1, C], mybir.dt.float32)
            nc.vector.reciprocal_approx_fast(out=s1[:, :], in_=tsq[:, :])
            nc.tensor.matmul(
                s_ps[:, 0, :], lhsT=ones_row[:, :], rhs=s1[:, :], start=True, stop=True
            )

            # Pass B: scale cached x, write out. Ramp the group size
            # (2,2,2,2,4,4, then 8s) so the first out-DMA launches right
            # after the scale is ready and the DMA queue never starves
            # while the first full-size mul runs.
            out_groups = (
                [(2 * h, 2) for h in range(4)]
                + [(8, 4), (12, 4)]
                + [(2 * G + j * G, G) for j in range(NG - 2)]
            )
            for t0, g in out_groups:
                ts_ = slice(t0, t0 + g)
                ot = outp.tile([P, g, C], mybir.dt.float32, tag="ot")
                nc.vector.tensor_mul(
                    ot, xc[:, ts_, :], s_ps[:, :, :].to_broadcast((P, g, C))
                )
                nc.sync.dma_start(out=yv[:, ts_, :], in_=ot)
    nc.compile()
    return nc


def _get_nc() -> bass.Bass:
    global _NC
    if _NC is None:
        _NC = _build()
    return _NC


def kernel(x) -> np.ndarray:
    x = np.asarray(x, dtype=np.float32)
    assert x.shape == (N, D), x.shape
    nc = _get_nc()
    in_maps = [
        {"x": np.ascontiguousarray(x[:, i * C : (i + 1) * C])} for i in range(NCORES)
    ]
    try:
        res = run_bass_kernel_spmd(nc, in_maps, core_ids=list(range(NCORES)))
    except Exception:
        # Transient NRT/device hiccups (e.g. a previous process's profiling
        # session left a core wedged) recover after a short pause.
        import time

        time.sleep(5)
        res = run_bass_kernel_spmd(nc, in_maps, core_ids=list(range(NCORES)))
    return np.concatenate([r["y"] for r in res.results], axis=1)



# revision 3
# speedup vs baseline: 1.6186x; 1.6186x over previous
"""Column-wise RMS normalization on 8 Trainium2 NeuronCores.

Computes y = x * rsqrt(sum(x*x, axis=0) + eps) for x [32768, 2048] f32.

Strategy: the tolerance (2e-2) admits fp16 I/O, so the host casts x to
fp16 and transposes it to [D, N]; each core owns 256 transposed rows
(original columns), two per partition ("(k p) t" layout). Each column
then lives entirely inside one partition with unit stride, so the
per-column sum-of-squares is a single fused DVE tensor_tensor_reduce
(square+accumulate) per chunk - no cross-partition matmul reduction,
no collectives - and the rsqrt scale is a per-partition scalar consumed
directly by tensor_scalar_mul.

The two column groups k=0/k=1 are software-pipelined: once k0's scale
is ready its scaled outputs stream out on the scalar (ACT) HWDGE ring
while k1's input still streams in on the sync ring, overlapping read
and write traffic. HBM traffic is 16MiB in + 16MiB out per core, half
the f32 floor.
"""

import numpy as np

import concourse.bacc as bacc
import concourse.bass as bass
import concourse.tile as tile
from concourse import mybir
from concourse.bass_utils import run_bass_kernel_spmd

N, D = 32768, 2048
EPS = 1e-6
NCORES = 8
R = D // NCORES  # 256 transposed rows (original columns) per core
P = 128          # partitions
K = R // P       # 2 column groups per core
T = N            # 32768 samples per column

# Chunk schedules along t (elements). Input tails ramp down so the last
# fused reduce feeding the scale is short; output heads ramp up so the
# first store launches right after the scale lands.
IN_CHUNKS = [4096] * 7 + [2048, 1024, 512, 256, 256]
OUT_CHUNKS = [256, 256, 512, 1024, 2048] + [4096] * 7
NCH = len(IN_CHUNKS)
assert sum(IN_CHUNKS) == T and sum(OUT_CHUNKS) == T

_NC = None


def _build() -> bass.Bass:
    nc = bacc.Bacc("TRN2", target_bir_lowering=False, enable_partition_id=False)
    x = nc.dram_tensor("x", [R, T], mybir.dt.float16, kind="ExternalInput")
    y = nc.dram_tensor("y", [R, T], mybir.dt.float16, kind="ExternalOutput")
    xv = x[:, :].rearrange("(k p) t -> p k t", k=K)
    yv = y[:, :].rearrange("(k p) t -> p k t", k=K)

    with tile.TileContext(nc) as tc:
        with (
            tc.tile_pool(name="cache", bufs=1) as cachep,
            tc.tile_pool(name="consts", bufs=1) as consts,
            tc.tile_pool(name="scr", bufs=2) as scrp,
            tc.tile_pool(name="outs", bufs=4) as outp,
        ):
            xc = cachep.tile([P, K, T], mybir.dt.float16)
            eps_t = consts.tile([P, 1], mybir.dt.float32)
            nc.vector.memset(eps_t, EPS)
            parts = consts.tile([P, K * NCH], mybir.dt.float32)
            u2 = consts.tile([P, K], mybir.dt.float32)
            t2 = consts.tile([P, K], mybir.dt.float32)
            s2 = consts.tile([P, K], mybir.dt.float32)

            def dma_in(k):
                t0 = 0
                for tc_ in IN_CHUNKS:
                    nc.sync.dma_start(
                        out=xc[:, k, t0 : t0 + tc_], in_=xv[:, k, t0 : t0 + tc_]
                    )
                    t0 += tc_

            def ttr(k, j, t0, tc_):
                # parts[:, k*NCH+j] = sum_t xc[:, k, t0:t0+tc_]**2
                # (scalar_tensor_tensor, not tensor_tensor_reduce: the latter
                # passes CoreSim but faults the exec unit on real TRN2)
                scr = scrp.tile([P, 4096], mybir.dt.float16, tag="scr")
                src = xc[:, k, t0 : t0 + tc_]
                nc.vector.scalar_tensor_tensor(
                    out=scr[:, :tc_],
                    in0=src,
                    scalar=1.0,
                    in1=src,
                    op0=mybir.AluOpType.mult,
                    op1=mybir.AluOpType.mult,
                    accum_out=parts[:, k * NCH + j : k * NCH + j + 1],
                )

            def scale(k):
                pv = parts[:, k * NCH : (k + 1) * NCH].rearrange(
                    "p (a j) -> p a j", a=1
                )
                nc.vector.reduce_sum(u2[:, k : k + 1], pv, axis=mybir.AxisListType.X)
                nc.scalar.activation(
                    out=t2[:, k : k + 1],
                    in_=u2[:, k : k + 1],
                    func=mybir.ActivationFunctionType.Sqrt,
                    bias=eps_t[:, :],
                    scale=1.0,
                )
                nc.vector.reciprocal_approx_fast(
                    out=s2[:, k : k + 1], in_=t2[:, k : k + 1]
                )

            def out_chunk(k, t0, tc_):
                ot = outp.tile([P, 4096], mybir.dt.float16, tag="ot")
                nc.vector.tensor_scalar_mul(
                    ot[:, :tc_], xc[:, k, t0 : t0 + tc_], s2[:, k : k + 1]
                )
                nc.scalar.dma_start(out=yv[:, k, t0 : t0 + tc_], in_=ot[:, :tc_])

            dma_in(0)
            dma_in(1)
            t0 = 0
            for j, tc_ in enumerate(IN_CHUNKS):
                ttr(0, j, t0, tc_)
                t0 += tc_
            scale(0)
            # k0 stores interleave with k1 reduces on the DVE queue
            t0o = t0r = 0
            for m in range(NCH):
                out_chunk(0, t0o, OUT_CHUNKS[m])
                t0o += OUT_CHUNKS[m]
                ttr(1, m, t0r, IN_CHUNKS[m])
                t0r += IN_CHUNKS[m]
            scale(1)
            t0o = 0
            for m in range(NCH):
                out_chunk(1, t0o, OUT_CHUNKS[m])
                t0o += OUT_CHUNKS[m]
    nc.compile()
    return nc


def _get_nc() -> bass.Bass:
    global _NC
    if _NC is None:
        _NC = _build()
    return _NC


def make_in_maps(x: np.ndarray) -> list[dict]:
    xt = np.ascontiguousarray(x.T.astype(np.float16))
    return [{"x": xt[i * R : (i + 1) * R]} for i in range(NCORES)]


def kernel(x) -> np.ndarray:
    x = np.asarray(x, dtype=np.float32)
    assert x.shape == (N, D), x.shape
    nc = _get_nc()
    in_maps = make_in_maps(x)
    try:
        res = run_bass_kernel_spmd(nc, in_maps, core_ids=list(range(NCORES)))
    except Exception:
        # Transient NRT/device hiccups (e.g. a previous process's profiling
        # session left a core wedged) recover after a short pause.
        import time

        time.sleep(5)
        res = run_bass_kernel_spmd(nc, in_maps, core_ids=list(range(NCORES)))
    yt = np.concatenate([r["y"] for r in res.results], axis=0)
    return yt.T.astype(np.float32)


# revision 6
# speedup vs baseline: 1.7959x; 1.1095x over previous
"""Column-wise RMS normalization on 8 Trainium2 NeuronCores.

Computes y = x * rsqrt(sum(x*x, axis=0) + eps) for x [32768, 2048] f32.

Strategy: the tolerance (2e-2) admits fp16 I/O, so the host casts x to
fp16 and transposes it to [D, N]; each core owns 256 transposed rows
(original columns), two per partition ("(k p) t" layout). Each column
then lives entirely inside one partition with unit stride, so the
per-column sum-of-squares needs no cross-partition reduction and the
rsqrt scale is a per-partition scalar consumed by tensor_scalar_mul.

The square+accumulate work is split between the vector engine
(scalar_tensor_tensor with accum_out; tensor_tensor_reduce faults real
TRN2 despite passing CoreSim) and the scalar engine (activation Square
with accum_out), sized so both engines hide under the DMA stream. The
two column groups k=0/k=1 are software-pipelined: k0's scaled outputs
stream out on the scalar HWDGE ring while k1's input still streams in
on the sync ring. HBM traffic is 16MiB in + 16MiB out per core, half
the f32 floor.
"""

import numpy as np

import concourse.bacc as bacc
import concourse.bass as bass
import concourse.tile as tile
from concourse import mybir
from concourse.bass_utils import run_bass_kernel_spmd

N, D = 32768, 2048
EPS = 1e-6
NCORES = 8
R = D // NCORES  # 256 transposed rows (original columns) per core
P = 128          # partitions
K = R // P       # 2 column groups per core
T = N            # 32768 samples per column

# Chunk schedules along t (elements). Input tails ramp down so the last
# square-accumulate feeding the scale is short; output heads ramp up so
# the first store launches right after the scale lands.
IN_CHUNKS = [4096] * 7 + [2048, 1024, 512, 256, 256]
DVE_IDX = {2, 5, 8, 9, 10, 11}  # vector engine's share; rest on scalar engine
OUT_CHUNKS = [512, 512, 1024, 2048, 4096, 8192, 8192, 8192]
NCH = len(IN_CHUNKS)
assert sum(IN_CHUNKS) == T and sum(OUT_CHUNKS) == T

_NC = None


def _build() -> bass.Bass:
    nc = bacc.Bacc("TRN2", target_bir_lowering=False, enable_partition_id=False)
    x = nc.dram_tensor("x", [R, T], mybir.dt.float16, kind="ExternalInput")
    y = nc.dram_tensor("y", [R, T], mybir.dt.float16, kind="ExternalOutput")
    xv = x[:, :].rearrange("(k p) t -> p k t", k=K)
    yv = y[:, :].rearrange("(k p) t -> p k t", k=K)

    with tile.TileContext(nc) as tc:
        with (
            tc.tile_pool(name="cache", bufs=1) as cachep,
            tc.tile_pool(name="consts", bufs=1) as consts,
            tc.tile_pool(name="scr", bufs=2) as scrp,
            tc.tile_pool(name="outs", bufs=2) as outp,
        ):
            xc = cachep.tile([P, K, T], mybir.dt.float16)
            eps_t = consts.tile([P, 1], mybir.dt.float32)
            nc.vector.memset(eps_t, EPS)
            parts = consts.tile([P, K * NCH], mybir.dt.float32)
            u2 = consts.tile([P, K], mybir.dt.float32)
            t2 = consts.tile([P, K], mybir.dt.float32)
            s2 = consts.tile([P, K], mybir.dt.float32)

            def dma_in(k):
                t0 = 0
                for tc_ in IN_CHUNKS:
                    nc.sync.dma_start(
                        out=xc[:, k, t0 : t0 + tc_], in_=xv[:, k, t0 : t0 + tc_]
                    )
                    t0 += tc_

            def a_chunk(k, j, t0, tc_):
                # parts[:, k*NCH+j] = sum_t xc[:, k, t0:t0+tc_]**2
                src = xc[:, k, t0 : t0 + tc_]
                acc = parts[:, k * NCH + j : k * NCH + j + 1]
                if j in DVE_IDX:
                    scr = scrp.tile([P, 4096], mybir.dt.float16, tag="scr")
                    nc.vector.scalar_tensor_tensor(
                        out=scr[:, :tc_],
                        in0=src,
                        scalar=1.0,
                        in1=src,
                        op0=mybir.AluOpType.mult,
                        op1=mybir.AluOpType.mult,
                        accum_out=acc,
                    )
                else:
                    scr = scrp.tile([P, 4096], mybir.dt.float16, tag="scra")
                    nc.scalar.activation(
                        out=scr[:, :tc_],
                        in_=src,
                        func=mybir.ActivationFunctionType.Square,
                        accum_out=acc,
                    )

            def scale(k):
                pv = parts[:, k * NCH : (k + 1) * NCH].rearrange(
                    "p (a j) -> p a j", a=1
                )
                nc.vector.reduce_sum(u2[:, k : k + 1], pv, axis=mybir.AxisListType.X)
                nc.scalar.activation(
                    out=t2[:, k : k + 1],
                    in_=u2[:, k : k + 1],
                    func=mybir.ActivationFunctionType.Sqrt,
                    bias=eps_t[:, :],
                    scale=1.0,
                )
                nc.vector.reciprocal_approx_fast(
                    out=s2[:, k : k + 1], in_=t2[:, k : k + 1]
                )

            def out_chunk(k, t0, tc_):
                ot = outp.tile([P, 8192], mybir.dt.float16, tag="ot")
                nc.vector.tensor_scalar_mul(
                    ot[:, :tc_], xc[:, k, t0 : t0 + tc_], s2[:, k : k + 1]
                )
                nc.scalar.dma_start(out=yv[:, k, t0 : t0 + tc_], in_=ot[:, :tc_])

            dma_in(0)
            dma_in(1)
            t0 = 0
            for j, tc_ in enumerate(IN_CHUNKS):
                a_chunk(0, j, t0, tc_)
                t0 += tc_
            scale(0)
            # k0 stores interleave with k1 square-accumulates
            in_off = [0]
            for tc_ in IN_CHUNKS[:-1]:
                in_off.append(in_off[-1] + tc_)
            t0o = 0
            for m in range(max(len(OUT_CHUNKS), NCH)):
                if m < len(OUT_CHUNKS):
                    out_chunk(0, t0o, OUT_CHUNKS[m])
                    t0o += OUT_CHUNKS[m]
                if m < NCH:
                    a_chunk(1, m, in_off[m], IN_CHUNKS[m])
            scale(1)
            t0o = 0
            for m in range(len(OUT_CHUNKS)):
                out_chunk(1, t0o, OUT_CHUNKS[m])
                t0o += OUT_CHUNKS[m]
    nc.compile()
    return nc


def _get_nc() -> bass.Bass:
    global _NC
    if _NC is None:
        _NC = _build()
    return _NC


def make_in_maps(x: np.ndarray) -> list[dict]:
    xt = np.ascontiguousarray(x.T.astype(np.float16))
    return [{"x": xt[i * R : (i + 1) * R]} for i in range(NCORES)]


def kernel(x) -> np.ndarray:
    x = np.asarray(x, dtype=np.float32)
    assert x.shape == (N, D), x.shape
    nc = _get_nc()
    in_maps = make_in_maps(x)
    try:
        res = run_bass_kernel_spmd(nc, in_maps, core_ids=list(range(NCORES)))
    except Exception:
        # Transient NRT/device hiccups (e.g. a previous process's profiling
        # session left a core wedged) recover after a short pause.
        import time

        time.sleep(5)
        res = run_bass_kernel_spmd(nc, in_maps, core_ids=list(range(NCORES)))
    yt = np.concatenate([r["y"] for r in res.results], axis=0)
    return yt.T.astype(np.float32)
